# revision 1
# baseline (speedup 1.0000x reference)
"""Causal self-attention (B=4, T=2048, D=1024, H=16, hd=64) on 8 TRN2 NeuronCores.

Sharding: core c handles batch b = c % 4 and head-half = c // 4 (8 heads each).
Each core computes, for its (batch, 8 heads):
    qkv projection -> causal attention -> partial output projection (yT).
Host gathers: y[b] = (yT[core b] + yT[core b+4]).T + b_proj.

Device design (per core):
  - transposed layouts: xT [D, T], qT/kT [hd-stacked, T], output yT [D, T]
  - S computed as S^T [k, q] tiles: lhsT = kT-slice, rhs = qT-slice; two heads
    row-packed on the PE at tile_position (0,0)/(64,0) (contraction = hd = 64)
  - softmax without max-subtraction (logits are small); exp on ScalarE with
    the 1/sqrt(hd) scale fused; causal masking via gpsimd affine_select on
    diagonal tiles only
  - V' = [V | 1] trick: wv gets a zero 65th column per head and bv' a 1.0, so
    each PV matmul (M=65, fp32r-legal dst partition 0) also produces the
    softmax denominator in row 64
  - normalize: DVE reciprocal (partition 64 -> 0/32), DMA partition-broadcast,
    DVE multiply into the proj input layout
  - all matmuls float32r (stored fp32 bits, full PE rate at N >= 256)
"""

import os
import sys
from contextlib import ExitStack

import numpy as np

sys.path.insert(0, "/opt/trn_rl_repo")

import concourse.bass as bass  # noqa: E402
import concourse.tile as tile  # noqa: E402
from concourse import bacc, mybir  # noqa: E402

f32 = mybir.dt.float32
f32r = mybir.dt.float32r
EXP = mybir.ActivationFunctionType.Exp

B, T, D = 4, 2048, 1024
H, HD = 16, 64
HDP = HD + 1       # 65: head dim + ones column
HPC = 8            # heads per core
NP = 4             # head pairs per core
NCORES = 8
TCH = 256          # phase-1 t-chunk width
NTCH = T // TCH    # 8
QCH = 512          # attention q-chunk width
NQCH = T // QCH    # 4
NKT = T // 128     # 16 k-tiles
VW = HPC * HDP     # 520: V' width
VH = VW // 2       # 260: V' half width (one matmul's N)


def _mm(nc, out, lhsT, rhs, **kw):
    nc.tensor.matmul(out, lhsT.bitcast(f32r), rhs.bitcast(f32r), **kw)


def build_program():
    nc = bacc.Bacc("TRN2", target_bir_lowering=False, debug=False)

    xT = nc.dram_tensor("xT", [D, T], f32r, kind="ExternalInput").ap()
    wqk = nc.dram_tensor("wqk", [D, 2 * HPC * HD], f32r, kind="ExternalInput").ap()
    wv = nc.dram_tensor("wv", [D, VW], f32r, kind="ExternalInput").ap()
    wp = nc.dram_tensor("wp", [HPC * HD, D], f32r, kind="ExternalInput").ap()
    bqk = nc.dram_tensor("bqk", [2 * HPC * HD, 1], f32, kind="ExternalInput").ap()
    bv = nc.dram_tensor("bv", [128, VW], f32, kind="ExternalInput").ap()
    sel = nc.dram_tensor("sel", [33, 128], f32r, kind="ExternalInput").ap()
    gainit = nc.dram_tensor("gainit", [128, 512], f32r, kind="ExternalInput").ap()
    yT = nc.dram_tensor("yT", [D, T], f32, kind="ExternalOutput").ap()

    with tile.TileContext(nc) as tc:
        with ExitStack() as ctx:
            _build(ctx, tc, xT, wqk, wv, wp, bqk, bv, sel, gainit, yT)
    nc.compile()
    return nc


def _build(ctx, tc, xT, wqk, wv, wp, bqk, bv, sel, gainit, yT):
    nc = tc.nc

    persist = ctx.enter_context(tc.tile_pool(name="persist", bufs=1))
    wqk_pool = ctx.enter_context(tc.tile_pool(name="wqk_pool", bufs=4))
    w16 = ctx.enter_context(tc.tile_pool(name="w16", bufs=1))
    xc_pool = ctx.enter_context(tc.tile_pool(name="xc_pool", bufs=1))
    es_pool = ctx.enter_context(tc.tile_pool(name="es_pool", bufs=3))
    pin_pool = ctx.enter_context(tc.tile_pool(name="pin_pool", bufs=2))
    small = ctx.enter_context(tc.tile_pool(name="small", bufs=6))

    p1ps = ctx.enter_context(tc.tile_pool(name="p1ps", bufs=2, space="PSUM"))
    s_ps_pool = ctx.enter_context(tc.tile_pool(name="s_ps_pool", bufs=2, space="PSUM"))
    pv_ps_pool = ctx.enter_context(tc.tile_pool(name="pv_ps_pool", bufs=2, space="PSUM"))

    # ---- persistent tensors ----
    qT = persist.tile([128, NP, T], f32r, tag="qT")     # [2 heads x 64 dims, pair, t]
    kT = persist.tile([128, NP, T], f32r, tag="kT")
    V = persist.tile([128, NKT, VW], f32r, tag="V")     # [t in tile, k-tile, h*65+d]

    # ---- constants / weights ----
    wqk_sb = []
    for s in range(4):
        w = wqk_pool.tile([128, 2, 2 * HPC * HD], f32r, name=f"wqk_sb{s}", tag="wqk")
        nc.sync.dma_start(
            out=w, in_=wqk[2 * s * 128:(2 * s + 2) * 128, :].rearrange(
                "(i p) m -> p i m", p=128))
        wqk_sb.append(w)
    wv_sb = w16.tile([128, 8, VW], f32r, tag="wv_sb")
    nc.sync.dma_start(out=wv_sb, in_=wv.rearrange("(d p) n -> p d n", p=128))

    bqk_sb = small.tile([128, 8], f32, tag="bqk_sb", bufs=1)
    nc.sync.dma_start(out=bqk_sb, in_=bqk.rearrange("(m p) o -> p (m o)", p=128))
    bv_sb = small.tile([128, VW], f32, tag="bv_sb", bufs=1)
    nc.sync.dma_start(out=bv_sb, in_=bv)
    sel_sb = small.tile([128, 128], f32r, tag="sel_sb", bufs=1)
    nc.sync.dma_start(out=sel_sb[0:33, :], in_=sel)
    nc.sync.dma_start(out=sel_sb[64:97, :], in_=sel)
    zreg = nc.gpsimd.to_reg(0.0)

    wp_sb = None  # loaded lazily after phase-1 starts

    # ================= phase 1: qkv projection (per t-chunk) =================
    def p1_chunk(tc_i):
        xc = xc_pool.tile([128, 8, TCH], f32r, name=f"xc{tc_i}", tag="xc")
        nc.sync.dma_start(
            out=xc,
            in_=xT.rearrange("(d p) t -> p d t", p=128)[:, :, tc_i * TCH:(tc_i + 1) * TCH])
        # q^T / k^T: out m-tile rows, t cols
        for mt in range(8):
            qk_ps = p1ps.tile([128, TCH], f32, name=f"qk_ps_{tc_i}_{mt}", tag="p1")
            for dt in range(8):
                _mm(nc, qk_ps, wqk_sb[dt // 2][:, dt % 2, mt * 128:(mt + 1) * 128],
                    xc[:, dt, :], start=(dt == 0), stop=(dt == 7))
            dest = qT if mt < 4 else kT
            nc.vector.tensor_scalar_add(
                dest[:, mt % 4, tc_i * TCH:(tc_i + 1) * TCH], qk_ps, bqk_sb[:, mt:mt + 1])
        # V': natural layout [t, h*65+d], two N=260 matmul groups per t-tile
        for tt in range(TCH // 128):
            t_idx = tc_i * (TCH // 128) + tt
            for g in range(2):
                v_ps = p1ps.tile([128, 512], f32, name=f"v_ps_{tc_i}_{tt}_{g}", tag="p1")
                for dt in range(8):
                    _mm(nc, v_ps[:, 0:VH], xc[:, dt, tt * 128:(tt + 1) * 128],
                        wv_sb[:, dt, g * VH:(g + 1) * VH],
                        start=(dt == 0), stop=(dt == 7))
                nc.vector.tensor_tensor(
                    out=V[:, t_idx, g * VH:(g + 1) * VH], in0=v_ps[:, 0:VH],
                    in1=bv_sb[:, g * VH:(g + 1) * VH], op=mybir.AluOpType.add)

    # ================= phase 2: attention for q-chunk j =================
    def make_gather(j):
        # sums rows live at partitions {0,32,64,96} (legal DVE bases); rest
        # memset to 1.0 so the batched in-place reciprocal stays finite
        gather = []
        for i in range(2):
            ga = small.tile([128, QCH], f32r, name=f"gather{j}_{i}", tag="ga", bufs=3)
            nc.sync.dma_start(out=ga, in_=gainit)
            gather.append(ga)
        return gather

    def attn_chunk(j, proj_in):
        gather = make_gather(j)
        for p in range(NP):
            attn_pair(j, p, proj_in, gather)
        attn_finish(j, proj_in, gather)

    def attn_finish(j, proj_in, gather):
        # batched reciprocal: DVE cost scales with free dim only, so one
        # in-place [128, 512] op covers 4 heads
        with nc.allow_low_precision(reason="fp32r denominators: rounding is benign"):
            nc.vector.reciprocal(gather[0], gather[0])
            nc.vector.reciprocal(gather[1], gather[1])
        for p in range(NP):
            ga = gather[p // 2]
            r0 = 64 * (p % 2)
            # partition-broadcast via PE: sel33 puts ga row r0 on partitions
            # 0..63 and row r0+32 on 64..127 (zero rows cancel the garbage)
            bc_ps = s_ps_pool.tile([128, QCH], f32, name=f"bcps_{j}_{p}", tag="s")
            _mm(nc, bc_ps, sel_sb[r0:r0 + 33, :], ga[r0:r0 + 33, :], start=True, stop=True)
            nc.vector.tensor_tensor(out=proj_in[:, p, :], in0=proj_in[:, p, :],
                                    in1=bc_ps, op=mybir.AluOpType.mult)

    def attn_pair(j, p, proj_in, gather):
        q0 = j * QCH
        if True:
            pvA = pv_ps_pool.tile([128, QCH], f32, name=f"pvA_{j}_{p}", tag="pv")
            pvB = pv_ps_pool.tile([128, QCH], f32, name=f"pvB_{j}_{p}", tag="pv")
            nkt = 4 * (j + 1)
            last = nkt - 1
            for kt in range(nkt):
                # o > 0 on diagonal tiles: columns [0:128*o) are fully masked,
                # so QKT/exp/PV all skip them (exact).
                o = max(0, kt - 4 * j)
                c0 = 128 * o
                W = QCH - c0
                s = s_ps_pool.tile([128, 2 * QCH], f32, name=f"s_{j}_{p}_{kt}", tag="s")
                _mm(nc, s[:, c0:QCH], kT[0:64, p, kt * 128:(kt + 1) * 128],
                    qT[0:64, p, q0 + c0:q0 + QCH],
                    start=True, stop=True, tile_position=(0, 0))
                _mm(nc, s[:, QCH + c0:2 * QCH], kT[64:128, p, kt * 128:(kt + 1) * 128],
                    qT[64:128, p, q0 + c0:q0 + QCH],
                    start=True, stop=True, tile_position=(64, 0))
                e = es_pool.tile([128, 2 * QCH], f32r, name=f"e_{j}_{p}_{kt}", tag="e")
                eA = e[:, c0:QCH]
                eB = e[:, QCH + c0:2 * QCH]
                if o == 0:
                    nc.scalar.activation(e, s, EXP, scale=0.125)
                else:
                    # one strided activation covering both heads' valid ranges
                    sv = s.rearrange("p (h q) -> p h q", h=2)[:, :, c0:QCH]
                    ev = e.rearrange("p (h q) -> p h q", h=2)[:, :, c0:QCH]
                    nc.scalar.activation(ev, sv, EXP, scale=0.125)
                if kt >= 4 * j:
                    # staircase mask within the remaining width: keep col >= kl
                    ev2 = e.rearrange("p (h q) -> p h q", h=2)[:, :, c0:QCH]
                    nc.gpsimd.affine_select(
                        ev2, ev2, pattern=[[0, 2], [1, W]],
                        compare_op=mybir.AluOpType.is_ge, fill=zreg,
                        base=0, channel_multiplier=-1)
                hA, hB = 2 * p, 2 * p + 1
                _mm(nc, pvA[0:HDP, c0:QCH], V[:, kt, hA * HDP:(hA + 1) * HDP], eA,
                    start=(kt == 0), stop=(kt == last))
                _mm(nc, pvB[0:HDP, c0:QCH], V[:, kt, hB * HDP:(hB + 1) * HDP], eB,
                    start=(kt == 0), stop=(kt == last))
            # Drain PV psum fast (frees the banks for the next pair):
            # U rows into proj_in (unnormalized) and the sums rows into the
            # per-chunk gather tile. Normalization happens once per chunk in
            # attn_finish, entirely off the PE-critical path.
            nc.vector.tensor_copy(out=proj_in[0:64, p, :], in_=pvA[0:64, :])
            nc.vector.tensor_copy(out=proj_in[64:128, p, :], in_=pvB[0:64, :])
            ga = gather[p // 2]
            r0 = 64 * (p % 2)
            nc.vector.tensor_copy(out=ga[r0:r0 + 1, :], in_=pvA[64:65, :])
            nc.vector.tensor_copy(out=ga[r0 + 32:r0 + 33, :], in_=pvB[64:65, :])

    # ================= phase 3: output projection for q-chunk j =================
    def proj_chunk(j, proj_in):
        for mt in range(8):
            y_ps = p1ps.tile([128, QCH], f32, name=f"y_{j}_{mt}", tag="p1")
            for p in range(NP):
                _mm(nc, y_ps, wp_sb[:, p, mt * 128:(mt + 1) * 128], proj_in[:, p, :],
                    start=(p == 0), stop=(p == NP - 1))
            y_sb = small.tile([128, QCH], f32, name=f"ysb_{j}_{mt}", tag="bc", bufs=1)
            nc.vector.tensor_copy(out=y_sb, in_=y_ps)
            nc.sync.dma_start(
                out=yT[mt * 128:(mt + 1) * 128, j * QCH:(j + 1) * QCH], in_=y_sb)

    # ================= emission: interleave =================
    def pin_tile(j):
        return pin_pool.tile([128, NP, QCH], f32r, name=f"pin{j}", tag="pin")

    # phase-1 chunk 2*j feeds attention chunk j's new k-tiles
    p1_chunk(0)
    p1_chunk(1)
    wp_sb = w16.tile([128, NP, D], f32r, tag="wp_sb")
    nc.sync.dma_start(out=wp_sb, in_=wp.rearrange("(k p) m -> p k m", p=128))
    pin0 = pin_tile(0)
    attn_chunk(0, pin0)
    proj_chunk(0, pin0)
    p1_chunk(2)
    p1_chunk(3)
    pin1 = pin_tile(1)
    attn_chunk(1, pin1)
    proj_chunk(1, pin1)
    p1_chunk(4)
    p1_chunk(5)
    pin2 = pin_tile(2)
    ga2 = make_gather(2)
    attn_pair(2, 0, pin2, ga2)
    p1_chunk(6)
    attn_pair(2, 1, pin2, ga2)
    p1_chunk(7)
    # tail: interleave remaining chunk-2 pairs with chunk-3 pairs so two
    # independent QKT/exp/PV streams keep PE and ACT dense
    pin3 = pin_tile(3)
    ga3 = make_gather(3)
    attn_pair(2, 2, pin2, ga2)
    attn_pair(3, 0, pin3, ga3)
    attn_pair(2, 3, pin2, ga2)
    attn_pair(3, 1, pin3, ga3)
    attn_finish(2, pin2, ga2)
    proj_chunk(2, pin2)
    attn_pair(3, 2, pin3, ga3)
    attn_pair(3, 3, pin3, ga3)
    attn_finish(3, pin3, ga3)
    proj_chunk(3, pin3)


# ======================= host side =======================

_NC_CACHE = None
LAST_RESULT = None


def _get_program():
    global _NC_CACHE
    if _NC_CACHE is None:
        _NC_CACHE = build_program()
    return _NC_CACHE


def shard_inputs(x, w_qkv, b_qkv, w_proj):
    x = np.asarray(x, dtype=np.float32)
    w_qkv = np.asarray(w_qkv, dtype=np.float32)
    b_qkv = np.asarray(b_qkv, dtype=np.float32)
    w_proj = np.asarray(w_proj, dtype=np.float32)
    sel_const = np.zeros((33, 128), dtype=np.float32)
    sel_const[0, 0:64] = 1.0
    sel_const[32, 64:128] = 1.0
    gainit_const = np.ones((128, 512), dtype=np.float32)
    in_maps = []
    for c in range(NCORES):
        b = c % B
        half = c // B
        hs = half * (HPC * HD)  # 512
        wq = w_qkv[:, 0 * D + hs:0 * D + hs + HPC * HD]
        wk = w_qkv[:, 1 * D + hs:1 * D + hs + HPC * HD]
        wv_ = w_qkv[:, 2 * D + hs:2 * D + hs + HPC * HD]
        bq = b_qkv[0 * D + hs:0 * D + hs + HPC * HD]
        bk = b_qkv[1 * D + hs:1 * D + hs + HPC * HD]
        bv_ = b_qkv[2 * D + hs:2 * D + hs + HPC * HD]
        # V' = [V | 1]: wv gets a zero 65th column per head; bv' a 1.0 there
        wvp = np.zeros((D, VW), dtype=np.float32)
        bvp = np.zeros((VW,), dtype=np.float32)
        for h in range(HPC):
            wvp[:, h * HDP:h * HDP + HD] = wv_[:, h * HD:(h + 1) * HD]
            bvp[h * HDP:h * HDP + HD] = bv_[h * HD:(h + 1) * HD]
            bvp[h * HDP + HD] = 1.0
        in_maps.append({
            "xT": np.ascontiguousarray(x[b].T),
            "wqk": np.ascontiguousarray(np.concatenate([wq, wk], axis=1)),
            "wv": wvp,
            "wp": np.ascontiguousarray(w_proj[hs:hs + HPC * HD, :]),
            "bqk": np.ascontiguousarray(np.concatenate([bq, bk])[:, None]),
            "bv": np.ascontiguousarray(np.broadcast_to(bvp[None, :], (128, VW))),
            "sel": sel_const,
            "gainit": gainit_const,
        })
    return in_maps


def kernel(x, w_qkv, b_qkv, w_proj, b_proj):
    global LAST_RESULT
    from concourse.bass_utils import run_bass_kernel_spmd

    nc = _get_program()
    in_maps = shard_inputs(x, w_qkv, b_qkv, w_proj)
    res = run_bass_kernel_spmd(nc, in_maps, list(range(NCORES)))
    LAST_RESULT = res
    b_proj = np.asarray(b_proj, dtype=np.float32)
    y = np.empty((B, T, D), dtype=np.float32)
    for b in range(B):
        yTfull = res.results[b]["yT"] + res.results[b + B]["yT"]
        y[b] = yTfull.T + b_proj[None, :]
    return y



# revision 5
# speedup vs baseline: 1.7325x; 1.7325x over previous
"""Causal self-attention (B=4, T=2048, D=1024, H=16, hd=64) on 8 TRN2 NeuronCores.

Sharding: core c handles batch b = c % 4 and head-half = c // 4 (8 heads each).
Each core computes, for its (batch, 8 heads):
    qkv projection -> causal attention -> partial output projection (yT).
Host gathers: y[b] = (yT[core b] + yT[core b+4]).T + b_proj.

Device design (per core), v2 (fp16):
  - everything fp16 on the PE (1 cyc/row, FWL weight loads); PSUM fp32
  - transposed layouts: xT [D, T], qT/kT [2heads x 64, pair, T], V [t, kt, 8h*64]
  - S^T [k, q] tiles: two heads row-packed at tile_position (0,0)/(64,0)
  - exp on ScalarE (scale fused); causal mask via gpsimd affine_select on
    diagonal tiles only
  - PV col-packed: head A -> psum rows 0:64 (tile_position (0,0)), head B ->
    rows 64:128 ((0,64)) - both matmuls run concurrently (one 512-cyc slot)
  - softmax denominator: DVE accumulates e tiles into a per-pair partial
    (fp16, 4x DVE mode); one col-packed pair of ones[128,64] matmuls reduces
    AND broadcasts the denominators into a [128,512] psum; DVE reciprocal ->
    bc; the PV psum drain is fused with the normalize multiply
  - emission interleaves p1/proj matmul "filler" units into the attention
    k-tile loop (engines are FIFO; emission order = execution order), with
    PV/denominator ops deferred by 2 steps so the PE never waits on exp
  - PE warmup matmuls + split initial DMAs (both HWDGE queues) cover the
    startup weight-load window and keep the HAM clock gate open
"""

import sys
from collections import deque
from contextlib import ExitStack

import numpy as np

sys.path.insert(0, "/opt/trn_rl_repo")

import concourse.bass as bass  # noqa: E402
import concourse.tile as tile  # noqa: E402
from concourse import bacc, mybir  # noqa: E402

f16 = mybir.dt.float16
f32 = mybir.dt.float32
EXP = mybir.ActivationFunctionType.Exp
ADD = mybir.AluOpType.add
MULT = mybir.AluOpType.mult

B, T, D = 4, 2048, 1024
H, HD = 16, 64
HPC = 8            # heads per core
NP = 4             # head pairs per core
NCORES = 8
TCH = 512          # p1 t-chunk width == attention q-chunk width
NCH = T // TCH     # 4
QCH = 512
NKT = T // 128     # 16 k-tiles


def build_program():
    nc = bacc.Bacc("TRN2", target_bir_lowering=False, debug=False)

    xT = nc.dram_tensor("xT", [D, T], f16, kind="ExternalInput").ap()
    wqk = nc.dram_tensor("wqk", [D, 2 * HPC * HD], f16, kind="ExternalInput").ap()
    wv = nc.dram_tensor("wv", [D, HPC * HD], f16, kind="ExternalInput").ap()
    wp = nc.dram_tensor("wp", [HPC * HD, D], f16, kind="ExternalInput").ap()
    bqk = nc.dram_tensor("bqk", [2 * HPC * HD, 1], f32, kind="ExternalInput").ap()
    bv = nc.dram_tensor("bv", [128, HPC * HD], f16, kind="ExternalInput").ap()
    ones = nc.dram_tensor("ones", [128, 64], f16, kind="ExternalInput").ap()
    yT = nc.dram_tensor("yT", [D, T], f16, kind="ExternalOutput").ap()

    with tile.TileContext(nc) as tc:
        with ExitStack() as ctx:
            _build(ctx, tc, xT, wqk, wv, wp, bqk, bv, ones, yT)
    nc.compile()
    return nc


def _build(ctx, tc, xT, wqk, wv, wp, bqk, bv, ones, yT):
    nc = tc.nc

    persist = ctx.enter_context(tc.tile_pool(name="persist", bufs=1))
    w_pool = ctx.enter_context(tc.tile_pool(name="w_pool", bufs=1))
    xc_pool = ctx.enter_context(tc.tile_pool(name="xc_pool", bufs=2))
    e_pool = ctx.enter_context(tc.tile_pool(name="e_pool", bufs=4))
    pa_pool = ctx.enter_context(tc.tile_pool(name="pa_pool", bufs=3))
    pin_pool = ctx.enter_context(tc.tile_pool(name="pin_pool", bufs=2))
    bc_pool = ctx.enter_context(tc.tile_pool(name="bc_pool", bufs=2))
    ysb_pool = ctx.enter_context(tc.tile_pool(name="ysb_pool", bufs=2))
    small = ctx.enter_context(tc.tile_pool(name="small", bufs=1))

    p1ps = ctx.enter_context(tc.tile_pool(name="p1ps", bufs=2, space="PSUM"))
    s_ps = ctx.enter_context(tc.tile_pool(name="s_ps", bufs=2, space="PSUM"))
    pv_ps = ctx.enter_context(tc.tile_pool(name="pv_ps", bufs=2, space="PSUM"))

    # ---- persistent tensors ----
    qT = persist.tile([128, NP, T], f16, tag="qT")      # [2 heads x 64 dims, pair, t]
    kT = persist.tile([128, NP, T], f16, tag="kT")
    V = persist.tile([128, NKT, HPC * HD], f16, tag="V")  # [t in tile, k-tile, h*64+d]

    # ---- initial DMAs, round-robin over both HWDGE queues (sync + scalar) ----
    dmae = [nc.sync, nc.scalar]
    ones_sb = small.tile([128, 64], f16, tag="ones_sb", bufs=1)
    nc.sync.dma_start(out=ones_sb, in_=ones)

    xTr = xT.rearrange("(d p) t -> p d t", p=128)

    def load_xc(c):
        xc = xc_pool.tile([128, 8, TCH], f16, name=f"xc{c}", tag="xc")
        nc.sync.dma_start(out=xc[:, 0:4, :], in_=xTr[:, 0:4, c * TCH:(c + 1) * TCH])
        nc.scalar.dma_start(out=xc[:, 4:8, :], in_=xTr[:, 4:8, c * TCH:(c + 1) * TCH])
        return xc

    xc0 = load_xc(0)
    wqk_sb = w_pool.tile([128, 8, 2 * HPC * HD], f16, tag="wqk_sb")
    for i in range(8):
        dmae[i % 2].dma_start(out=wqk_sb[:, i, :], in_=wqk[i * 128:(i + 1) * 128, :])
    wv_sb = w_pool.tile([128, 8, HPC * HD], f16, tag="wv_sb")
    for i in range(8):
        dmae[i % 2].dma_start(out=wv_sb[:, i, :], in_=wv[i * 128:(i + 1) * 128, :])
    bqk_sb = small.tile([128, 8], f32, tag="bqk_sb", bufs=1)
    nc.sync.dma_start(out=bqk_sb, in_=bqk.rearrange("(m p) o -> p (m o)", p=128))
    bv_sb = small.tile([128, HPC * HD], f16, tag="bv_sb", bufs=1)
    nc.sync.dma_start(out=bv_sb, in_=bv)
    xc1 = load_xc(1)
    wp_sb = w_pool.tile([128, NP, D], f16, tag="wp_sb")
    for i in range(NP):
        dmae[i % 2].dma_start(out=wp_sb[:, i, :], in_=wp[i * 128:(i + 1) * 128, :])

    zreg = nc.gpsimd.to_reg(0.0)

    # ---- PE warmup: keep the PE busy (and the HAM gate open) during the
    # initial weight DMA; writes a never-read scratch psum ----
    scr = p1ps.tile([128, TCH], f32, name="warm", tag="p1")
    for i in range(72):
        nc.tensor.matmul(scr[0:64, 0:64], ones_sb, ones_sb, start=True, stop=True)

    # ================= phase 1 units (qkv projection) =================
    def p1_qk_unit(c, mt, xc):
        qk_ps = p1ps.tile([128, TCH], f32, name=f"qk{c}_{mt}", tag="p1")
        for dt in range(8):
            nc.tensor.matmul(qk_ps, wqk_sb[:, dt, mt * 128:(mt + 1) * 128],
                             xc[:, dt, :], start=(dt == 0), stop=(dt == 7))
        dest = qT if mt < 4 else kT
        nc.vector.tensor_scalar_add(
            dest[:, mt % 4, c * TCH:(c + 1) * TCH], qk_ps, bqk_sb[:, mt:mt + 1])

    def p1_v_unit(c, tt, xc):
        v_ps = p1ps.tile([128, 512], f32, name=f"v{c}_{tt}", tag="p1")
        for dt in range(8):
            nc.tensor.matmul(v_ps, xc[:, dt, tt * 128:(tt + 1) * 128],
                             wv_sb[:, dt, :], start=(dt == 0), stop=(dt == 7))
        nc.vector.tensor_tensor(out=V[:, c * 4 + tt, :], in0=v_ps, in1=bv_sb, op=ADD)

    def p1_units(c, xc):
        units = [(lambda c=c, mt=mt, xc=xc: p1_qk_unit(c, mt, xc)) for mt in range(8)]
        units += [(lambda c=c, tt=tt, xc=xc: p1_v_unit(c, tt, xc)) for tt in range(4)]
        return units

    # ================= phase 3 unit (output projection) =================
    def proj_unit(j, pin, mt):
        y_ps = p1ps.tile([128, QCH], f32, name=f"y{j}_{mt}", tag="p1")
        for p in range(NP):
            nc.tensor.matmul(y_ps, wp_sb[:, p, mt * 128:(mt + 1) * 128],
                             pin[:, p, :], start=(p == 0), stop=(p == NP - 1))
        y_sb = ysb_pool.tile([128, QCH], f16, name=f"ysb{j}_{mt}", tag="ysb")
        nc.vector.tensor_copy(out=y_sb, in_=y_ps)
        nc.sync.dma_start(
            out=yT[mt * 128:(mt + 1) * 128, j * QCH:(j + 1) * QCH], in_=y_sb)

    def proj_units(j, pin):
        return [(lambda j=j, pin=pin, mt=mt: proj_unit(j, pin, mt)) for mt in range(8)]

    # ================= phase 2: attention =================
    # deferred: PE-op closures executed with a 2-step lag so the PE (FIFO)
    # never queues a matmul whose exp input isn't ready yet.
    deferred = deque()

    def attn_chunk(j, fillers, prefetch=None):
        if prefetch is not None:
            prefetch()
        pin = pin_pool.tile([128, NP, QCH], f16, name=f"pin{j}", tag="pin")
        nkt = 4 * (j + 1)
        q0 = j * QCH
        nsteps = NP * nkt
        fill_per = len(fillers) / nsteps
        fill_acc = 0.0
        for p in range(NP):
            partial = pa_pool.tile([128, 2 * QCH], f16, name=f"pa{j}_{p}", tag="pa")
            pv = pv_ps.tile([128, QCH], f32, name=f"pv{j}_{p}", tag="pv")
            for kt in range(nkt):
                o = max(0, kt - 4 * j)
                c0 = 128 * o
                W = QCH - c0
                s = s_ps.tile([128, 2 * QCH], f32, name=f"s{j}_{p}_{kt}", tag="s")
                nc.tensor.matmul(
                    s[:, c0:QCH], kT[0:64, p, kt * 128:(kt + 1) * 128],
                    qT[0:64, p, q0 + c0:q0 + QCH],
                    start=True, stop=True, tile_position=(0, 0))
                nc.tensor.matmul(
                    s[:, QCH + c0:2 * QCH], kT[64:128, p, kt * 128:(kt + 1) * 128],
                    qT[64:128, p, q0 + c0:q0 + QCH],
                    start=True, stop=True, tile_position=(64, 0))
                e = e_pool.tile([128, 2 * QCH], f16, name=f"e{j}_{p}_{kt}", tag="e")
                if o == 0:
                    nc.scalar.activation(e, s, EXP, scale=0.125)
                else:
                    sv = s.rearrange("p (h q) -> p h q", h=2)[:, :, c0:QCH]
                    ev = e.rearrange("p (h q) -> p h q", h=2)[:, :, c0:QCH]
                    nc.scalar.activation(ev, sv, EXP, scale=0.125)
                if kt >= 4 * j:
                    # staircase mask within the remaining width: keep col >= k
                    ev2 = e.rearrange("p (h q) -> p h q", h=2)[:, :, c0:QCH]
                    nc.gpsimd.affine_select(
                        ev2, ev2, pattern=[[0, 2], [1, W]],
                        compare_op=mybir.AluOpType.is_ge, fill=zreg,
                        base=0, channel_multiplier=-1)
                # denominator partials (DVE, fp16 4x mode)
                if kt == 0:
                    nc.vector.tensor_copy(out=partial, in_=e)
                elif o == 0:
                    nc.vector.tensor_tensor(out=partial, in0=partial, in1=e, op=ADD)
                else:
                    pview = partial.rearrange("p (h q) -> p h q", h=2)[:, :, c0:QCH]
                    eview = e.rearrange("p (h q) -> p h q", h=2)[:, :, c0:QCH]
                    nc.vector.tensor_tensor(out=pview, in0=pview, in1=eview, op=ADD)

                def emit_pv(p=p, kt=kt, c0=c0, e=e, pv=pv, last=(kt == nkt - 1)):
                    nc.tensor.matmul(
                        pv[0:64, c0:QCH], V[:, kt, (2 * p) * 64:(2 * p + 1) * 64],
                        e[:, c0:QCH], start=(kt == 0), stop=last)
                    nc.tensor.matmul(
                        pv[64:128, c0:QCH], V[:, kt, (2 * p + 1) * 64:(2 * p + 2) * 64],
                        e[:, QCH + c0:2 * QCH], start=(kt == 0), stop=last)

                deferred.append(emit_pv)
                while len(deferred) > 2:
                    deferred.popleft()()
                fill_acc += fill_per
                while fillers and fill_acc >= 1.0:
                    fillers.popleft()()
                    fill_acc -= 1.0

            def emit_fin(j=j, p=p, partial=partial, pv=pv, pin=pin):
                # ones[128,64] matmuls: reduce the partial over k AND
                # broadcast the per-head denominators into all 64 rows
                dn = p1ps.tile([128, QCH], f32, name=f"dn{j}_{p}", tag="p1")
                nc.tensor.matmul(dn[0:64, :], ones_sb, partial[:, 0:QCH],
                                 start=True, stop=True)
                nc.tensor.matmul(dn[64:128, :], ones_sb, partial[:, QCH:2 * QCH],
                                 start=True, stop=True)
                bc = bc_pool.tile([128, QCH], f16, name=f"bc{j}_{p}", tag="bc")
                with nc.allow_low_precision(reason="fp16 softmax denominators"):
                    nc.vector.reciprocal(bc, dn)
                # fused PV-psum drain + normalize
                nc.vector.tensor_tensor(out=pin[:, p, :], in0=pv, in1=bc, op=MULT)

            deferred.append(emit_fin)
        while fillers:
            fillers.popleft()()
        return pin

    def interleave(a, b):
        out = []
        la, lb = list(a), list(b)
        n = max(len(la), len(lb))
        for i in range(n):
            if i < len(la):
                out.append(la[i])
            if i < len(lb):
                out.append(lb[i])
        return out

    # ================= emission =================
    for mt in range(8):
        p1_qk_unit(0, mt, xc0)
    for tt in range(4):
        p1_v_unit(0, tt, xc0)

    xc_next = {}

    def prefetch(c):
        def go():
            xc_next[c] = load_xc(c)
        return go

    pin0 = attn_chunk(0, deque(p1_units(1, xc1)), prefetch=prefetch(2))
    pin1 = attn_chunk(
        1, deque(interleave(p1_units(2, xc_next[2]), proj_units(0, pin0))),
        prefetch=prefetch(3))
    pin2 = attn_chunk(
        2, deque(interleave(p1_units(3, xc_next[3]), proj_units(1, pin1))))
    pin3 = attn_chunk(3, deque(proj_units(2, pin2)))
    while deferred:
        deferred.popleft()()
    for u in proj_units(3, pin3):
        u()


# ======================= host side =======================

_NC_CACHE = None
LAST_RESULT = None


def _get_program():
    global _NC_CACHE
    if _NC_CACHE is None:
        _NC_CACHE = build_program()
    return _NC_CACHE


def shard_inputs(x, w_qkv, b_qkv, w_proj):
    x = np.asarray(x, dtype=np.float32)
    w_qkv = np.asarray(w_qkv, dtype=np.float32)
    b_qkv = np.asarray(b_qkv, dtype=np.float32)
    w_proj = np.asarray(w_proj, dtype=np.float32)
    ones_const = np.ones((128, 64), dtype=np.float16)
    in_maps = []
    for c in range(NCORES):
        b = c % B
        half = c // B
        hs = half * (HPC * HD)  # 512
        wq = w_qkv[:, 0 * D + hs:0 * D + hs + HPC * HD]
        wk = w_qkv[:, 1 * D + hs:1 * D + hs + HPC * HD]
        wv_ = w_qkv[:, 2 * D + hs:2 * D + hs + HPC * HD]
        bq = b_qkv[0 * D + hs:0 * D + hs + HPC * HD]
        bk = b_qkv[1 * D + hs:1 * D + hs + HPC * HD]
        bv_ = b_qkv[2 * D + hs:2 * D + hs + HPC * HD]
        in_maps.append({
            "xT": np.ascontiguousarray(x[b].T).astype(np.float16),
            "wqk": np.ascontiguousarray(
                np.concatenate([wq, wk], axis=1)).astype(np.float16),
            "wv": np.ascontiguousarray(wv_).astype(np.float16),
            "wp": np.ascontiguousarray(w_proj[hs:hs + HPC * HD, :]).astype(np.float16),
            "bqk": np.ascontiguousarray(
                np.concatenate([bq, bk])[:, None]).astype(np.float32),
            "bv": np.ascontiguousarray(
                np.broadcast_to(bv_[None, :], (128, HPC * HD))).astype(np.float16),
            "ones": ones_const,
        })
    return in_maps


def kernel(x, w_qkv, b_qkv, w_proj, b_proj):
    global LAST_RESULT
    from concourse.bass_utils import run_bass_kernel_spmd

    nc = _get_program()
    in_maps = shard_inputs(x, w_qkv, b_qkv, w_proj)
    res = run_bass_kernel_spmd(nc, in_maps, list(range(NCORES)))
    LAST_RESULT = res
    b_proj = np.asarray(b_proj, dtype=np.float32)
    y = np.empty((B, T, D), dtype=np.float32)
    for b in range(B):
        yTfull = (res.results[b]["yT"].astype(np.float32)
                  + res.results[b + B]["yT"].astype(np.float32))
        y[b] = yTfull.T + b_proj[None, :]
    return y


# revision 9
# speedup vs baseline: 1.8325x; 1.0577x over previous
"""Causal self-attention (B=4, T=2048, D=1024, H=16, hd=64) on 8 TRN2 NeuronCores.

Sharding: core c handles batch b = c % 4 and head-half = c // 4 (8 heads each).
Each core computes, for its (batch, 8 heads):
    qkv projection -> causal attention -> partial output projection (yT).
Host gathers: y[b] = (yT[core b] + yT[core b+4]).T + b_proj.

Device design (per core), v2 (fp16):
  - everything fp16 on the PE (1 cyc/row, FWL weight loads); PSUM fp32
  - transposed layouts: xT [D, T], qT/kT [2heads x 64, pair, T], V [t, kt, 8h*64]
  - S^T [k, q] tiles: two heads row-packed at tile_position (0,0)/(64,0)
  - exp on ScalarE (scale fused); causal mask via gpsimd affine_select on
    diagonal tiles only
  - PV col-packed: head A -> psum rows 0:64 (tile_position (0,0)), head B ->
    rows 64:128 ((0,64)) - both matmuls run concurrently (one 512-cyc slot)
  - softmax denominator: DVE accumulates e tiles into a per-pair partial
    (fp16, 4x DVE mode); one col-packed pair of ones[128,64] matmuls reduces
    AND broadcasts the denominators into a [128,512] psum; DVE reciprocal ->
    bc; the PV psum drain is fused with the normalize multiply
  - emission interleaves p1/proj matmul "filler" units into the attention
    k-tile loop (engines are FIFO; emission order = execution order), with
    PV/denominator ops deferred by 2 steps so the PE never waits on exp
  - PE warmup matmuls + split initial DMAs (both HWDGE queues) cover the
    startup weight-load window and keep the HAM clock gate open
"""

import sys
from collections import deque
from contextlib import ExitStack

import numpy as np

sys.path.insert(0, "/opt/trn_rl_repo")

import concourse.bass as bass  # noqa: E402
import concourse.tile as tile  # noqa: E402
from concourse import bacc, mybir  # noqa: E402

f16 = mybir.dt.float16
bf16 = mybir.dt.bfloat16
f32 = mybir.dt.float32
EXP = mybir.ActivationFunctionType.Exp
ADD = mybir.AluOpType.add
MULT = mybir.AluOpType.mult

B, T, D = 4, 2048, 1024
H, HD = 16, 64
HPC = 8            # heads per core
NP = 4             # head pairs per core
NCORES = 8
TCH = 512          # p1 t-chunk width == attention q-chunk width
NCH = T // TCH     # 4
QCH = 512
NKT = T // 128     # 16 k-tiles


def build_program():
    nc = bacc.Bacc("TRN2", target_bir_lowering=False, debug=False)

    xT = nc.dram_tensor("xT", [D, T], f16, kind="ExternalInput").ap()
    wqk = nc.dram_tensor("wqk", [D, 2 * HPC * HD], f16, kind="ExternalInput").ap()
    wv = nc.dram_tensor("wv", [D, HPC * HD], f16, kind="ExternalInput").ap()
    wp = nc.dram_tensor("wp", [HPC * HD, D], f16, kind="ExternalInput").ap()
    bqk = nc.dram_tensor("bqk", [2 * HPC * HD, 1], f32, kind="ExternalInput").ap()
    bv = nc.dram_tensor("bv", [128, HPC * HD], f16, kind="ExternalInput").ap()
    yT = nc.dram_tensor("yT", [D, T], f16, kind="ExternalOutput").ap()

    with tile.TileContext(nc) as tc:
        with ExitStack() as ctx:
            _build(ctx, tc, xT, wqk, wv, wp, bqk, bv, yT)
    nc.compile()
    return nc


def _build(ctx, tc, xT, wqk, wv, wp, bqk, bv, yT):
    nc = tc.nc

    persist = ctx.enter_context(tc.tile_pool(name="persist", bufs=1))
    w_pool = ctx.enter_context(tc.tile_pool(name="w_pool", bufs=1))
    xc_pool = ctx.enter_context(tc.tile_pool(name="xc_pool", bufs=2))
    e_pool = ctx.enter_context(tc.tile_pool(name="e_pool", bufs=4))
    pa_pool = ctx.enter_context(tc.tile_pool(name="pa_pool", bufs=4))
    pin_pool = ctx.enter_context(tc.tile_pool(name="pin_pool", bufs=4))
    bc_pool = ctx.enter_context(tc.tile_pool(name="bc_pool", bufs=2))
    ysb_pool = ctx.enter_context(tc.tile_pool(name="ysb_pool", bufs=2))
    small = ctx.enter_context(tc.tile_pool(name="small", bufs=1))

    p1ps = ctx.enter_context(tc.tile_pool(name="p1ps", bufs=2, space="PSUM"))
    s_ps = ctx.enter_context(tc.tile_pool(name="s_ps", bufs=2, space="PSUM"))
    pv_ps = ctx.enter_context(tc.tile_pool(name="pv_ps", bufs=2, space="PSUM"))

    # ---- persistent tensors ----
    qT = persist.tile([128, NP, T], f16, tag="qT")      # [2 heads x 64 dims, pair, t]
    kT = persist.tile([128, NP, T], f16, tag="kT")
    V = persist.tile([128, NKT, HPC * HD], bf16, tag="V")  # [t in tile, k-tile, h*64+d]

    # ---- initial DMAs, round-robin over both HWDGE queues (sync + scalar) ----
    dmae = [nc.sync, nc.scalar]
    ones_sb = small.tile([128, 64], bf16, tag="ones_sb", bufs=1)
    nc.vector.memset(ones_sb, 1.0)

    xTr = xT.rearrange("(d p) t -> p d t", p=128)

    def load_xc(c):
        xc = xc_pool.tile([128, 8, TCH], f16, name=f"xc{c}", tag="xc")
        nc.sync.dma_start(out=xc[:, 0:4, :], in_=xTr[:, 0:4, c * TCH:(c + 1) * TCH])
        nc.scalar.dma_start(out=xc[:, 4:8, :], in_=xTr[:, 4:8, c * TCH:(c + 1) * TCH])
        return xc

    xc0 = load_xc(0)
    wqk_sb = w_pool.tile([128, 8, 2 * HPC * HD], f16, tag="wqk_sb")
    for i in range(8):
        dmae[i % 2].dma_start(out=wqk_sb[:, i, :], in_=wqk[i * 128:(i + 1) * 128, :])
    wv_sb = w_pool.tile([128, 8, HPC * HD], f16, tag="wv_sb")
    for i in range(8):
        dmae[i % 2].dma_start(out=wv_sb[:, i, :], in_=wv[i * 128:(i + 1) * 128, :])
    bqk_sb = small.tile([128, 8], f32, tag="bqk_sb", bufs=1)
    nc.sync.dma_start(out=bqk_sb, in_=bqk.rearrange("(m p) o -> p (m o)", p=128))
    bv_sb = small.tile([128, HPC * HD], f16, tag="bv_sb", bufs=1)
    nc.sync.dma_start(out=bv_sb, in_=bv)
    xc1 = load_xc(1)
    wp_sb = w_pool.tile([128, NP, D], f16, tag="wp_sb")
    for i in range(NP):
        dmae[i % 2].dma_start(out=wp_sb[:, i, :], in_=wp[i * 128:(i + 1) * 128, :])

    zreg = nc.gpsimd.to_reg(0.0)

    # ---- PE warmup: keep the PE busy (and the HAM gate open) during the
    # initial weight DMA; writes a never-read scratch psum ----
    scr = p1ps.tile([128, TCH], f32, name="warm", tag="p1")
    for i in range(104):
        nc.tensor.matmul(scr[0:64, 0:64], ones_sb, ones_sb, start=True, stop=True)

    # ================= phase 1 units (qkv projection) =================
    def p1_qk_unit(c, mt, xc):
        qk_ps = p1ps.tile([128, TCH], f32, name=f"qk{c}_{mt}", tag="p1")
        for dt in range(8):
            nc.tensor.matmul(qk_ps, wqk_sb[:, dt, mt * 128:(mt + 1) * 128],
                             xc[:, dt, :], start=(dt == 0), stop=(dt == 7))
        dest = qT if mt < 4 else kT
        nc.vector.tensor_scalar_add(
            dest[:, mt % 4, c * TCH:(c + 1) * TCH], qk_ps, bqk_sb[:, mt:mt + 1])

    def p1_v_unit(c, tt, xc):
        v_ps = p1ps.tile([128, 512], f32, name=f"v{c}_{tt}", tag="p1")
        for dt in range(8):
            nc.tensor.matmul(v_ps, xc[:, dt, tt * 128:(tt + 1) * 128],
                             wv_sb[:, dt, :], start=(dt == 0), stop=(dt == 7))
        nc.vector.tensor_tensor(out=V[:, c * 4 + tt, :], in0=v_ps, in1=bv_sb, op=ADD)

    def p1_units(c, xc):
        units = [(lambda c=c, mt=mt, xc=xc: p1_qk_unit(c, mt, xc)) for mt in range(8)]
        units += [(lambda c=c, tt=tt, xc=xc: p1_v_unit(c, tt, xc)) for tt in range(4)]
        return units

    # ================= phase 3 unit (output projection) =================
    def proj_unit(j, pin, mt):
        y_ps = p1ps.tile([128, QCH], f32, name=f"y{j}_{mt}", tag="p1")
        for p in range(NP):
            nc.tensor.matmul(y_ps, wp_sb[:, p, mt * 128:(mt + 1) * 128],
                             pin[:, p, :], start=(p == 0), stop=(p == NP - 1))
        y_sb = ysb_pool.tile([128, QCH], f16, name=f"ysb{j}_{mt}", tag="ysb")
        nc.vector.tensor_copy(out=y_sb, in_=y_ps)
        nc.sync.dma_start(
            out=yT[mt * 128:(mt + 1) * 128, j * QCH:(j + 1) * QCH], in_=y_sb)

    def interleave(a, b):
        out = []
        la, lb = list(a), list(b)
        n = max(len(la), len(lb))
        for i in range(n):
            if i < len(la):
                out.append(la[i])
            if i < len(lb):
                out.append(lb[i])
        return out

    def proj_units(j, pin):
        return [(lambda j=j, pin=pin, mt=mt: proj_unit(j, pin, mt)) for mt in range(8)]

    # ================= phase 2: attention =================
    # deferred: PE-op closures executed with a 2-step lag so the PE (FIFO)
    # never queues a matmul whose exp input isn't ready yet.
    deferred = deque()
    fillers = deque()
    fill_state = {"acc": 0.0}

    def attn_pair(j, p, pin, rate):
        nkt = 4 * (j + 1)
        q0 = j * QCH
        partial = pa_pool.tile([128, 2 * QCH], bf16, name=f"pa{j}_{p}", tag="pa")
        pv = pv_ps.tile([128, QCH], f32, name=f"pv{j}_{p}", tag="pv")
        for kt in range(nkt):
            o = max(0, kt - 4 * j)
            c0 = 128 * o
            W = QCH - c0
            s = s_ps.tile([128, 2 * QCH], f32, name=f"s{j}_{p}_{kt}", tag="s")
            nc.tensor.matmul(
                s[:, c0:QCH], kT[0:64, p, kt * 128:(kt + 1) * 128],
                qT[0:64, p, q0 + c0:q0 + QCH],
                start=True, stop=True, tile_position=(0, 0))
            nc.tensor.matmul(
                s[:, QCH + c0:2 * QCH], kT[64:128, p, kt * 128:(kt + 1) * 128],
                qT[64:128, p, q0 + c0:q0 + QCH],
                start=True, stop=True, tile_position=(64, 0))
            e = e_pool.tile([128, 2 * QCH], bf16, name=f"e{j}_{p}_{kt}", tag="e")
            if o == 0:
                nc.scalar.activation(e, s, EXP, scale=0.125)
            else:
                sv = s.rearrange("p (h q) -> p h q", h=2)[:, :, c0:QCH]
                ev = e.rearrange("p (h q) -> p h q", h=2)[:, :, c0:QCH]
                nc.scalar.activation(ev, sv, EXP, scale=0.125)
            if kt >= 4 * j:
                # staircase mask within the remaining width: keep col >= k
                ev2 = e.rearrange("p (h q) -> p h q", h=2)[:, :, c0:QCH]
                nc.gpsimd.affine_select(
                    ev2, ev2, pattern=[[0, 2], [1, W]],
                    compare_op=mybir.AluOpType.is_ge, fill=zreg,
                    base=0, channel_multiplier=-1)
            # denominator partials (DVE)
            if kt == 0:
                nc.vector.tensor_copy(out=partial, in_=e)
            elif o == 0:
                nc.vector.tensor_tensor(out=partial, in0=partial, in1=e, op=ADD)
            else:
                pview = partial.rearrange("p (h q) -> p h q", h=2)[:, :, c0:QCH]
                eview = e.rearrange("p (h q) -> p h q", h=2)[:, :, c0:QCH]
                nc.vector.tensor_tensor(out=pview, in0=pview, in1=eview, op=ADD)

            def emit_pv(p=p, kt=kt, c0=c0, e=e, pv=pv, last=(kt == nkt - 1)):
                nc.tensor.matmul(
                    pv[0:64, c0:QCH], V[:, kt, (2 * p) * 64:(2 * p + 1) * 64],
                    e[:, c0:QCH], start=(kt == 0), stop=last)
                nc.tensor.matmul(
                    pv[64:128, c0:QCH], V[:, kt, (2 * p + 1) * 64:(2 * p + 2) * 64],
                    e[:, QCH + c0:2 * QCH], start=(kt == 0), stop=last)

            deferred.append(emit_pv)
            while len(deferred) > 2:
                deferred.popleft()()
            fill_state["acc"] += rate
            while fillers and fill_state["acc"] >= 1.0:
                fillers.popleft()()
                fill_state["acc"] -= 1.0

        def emit_fin(j=j, p=p, partial=partial, pv=pv, pin=pin):
            # ones[128,64] matmuls: reduce the partial over k AND broadcast
            # the per-head denominators into all 64 rows of a psum bank
            dn = p1ps.tile([128, QCH], f32, name=f"dn{j}_{p}", tag="p1")
            nc.tensor.matmul(dn[0:64, :], ones_sb, partial[:, 0:QCH],
                             start=True, stop=True)
            nc.tensor.matmul(dn[64:128, :], ones_sb, partial[:, QCH:2 * QCH],
                             start=True, stop=True)
            bc = bc_pool.tile([128, QCH], f32, name=f"bc{j}_{p}", tag="bc")
            nc.vector.reciprocal_approx_fast(out=bc, in_=dn)
            # fused PV-psum drain + normalize
            nc.vector.tensor_tensor(out=pin[:, p, :], in0=pv, in1=bc, op=MULT)

        deferred.append(emit_fin)

    # ================= emission =================
    for mt in range(8):
        p1_qk_unit(0, mt, xc0)
    for tt in range(4):
        p1_v_unit(0, tt, xc0)

    xc_next = {}

    def prefetch(c):
        xc_next[c] = load_xc(c)

    def new_pin(j):
        return pin_pool.tile([128, NP, QCH], f16, name=f"pin{j}", tag="pin")

    # --- chunk 0: fillers = p1(c1) ---
    prefetch(2)
    pin0 = new_pin(0)
    fillers.extend(p1_units(1, xc1))
    fill_state["acc"] = 0.0
    for p in range(NP):
        attn_pair(0, p, pin0, 12.0 / 16.0)
    while fillers:
        fillers.popleft()()

    # --- chunk 1: fillers = p1 qk(c2), V(c2), qk(c3) ---
    prefetch(3)
    pin1 = new_pin(1)
    xc2, xc3 = xc_next[2], xc_next[3]
    fillers.extend([(lambda mt=mt: p1_qk_unit(2, mt, xc2)) for mt in range(8)])
    fillers.extend([(lambda tt=tt: p1_v_unit(2, tt, xc2)) for tt in range(4)])
    fillers.extend([(lambda mt=mt: p1_qk_unit(3, mt, xc3)) for mt in range(8)])
    fill_state["acc"] = 0.0
    for p in range(NP):
        attn_pair(1, p, pin1, 20.0 / 32.0)
    while fillers:
        fillers.popleft()()

    # --- chunks 2+3 interleaved by pair: exp-heavy chunk-3 pairs overlap
    # proj filler work instead of piling up at the end ---
    pin2 = new_pin(2)
    pin3 = new_pin(3)
    fillers.extend([(lambda tt=tt: p1_v_unit(3, tt, xc3)) for tt in range(4)])
    fillers.extend(interleave(proj_units(0, pin0), proj_units(1, pin1)))
    fill_state["acc"] = 0.0
    region = [(2, 0), (3, 0), (2, 1), (3, 1), (2, 2), (3, 2), (2, 3), (3, 3)]
    for idx, (j, p) in enumerate(region):
        if (j, p) == (3, 3):
            fillers.extend(proj_units(2, pin2))
            fill_state["acc"] = -2.0 * (len(fillers) / 16.0)
            rate = len(fillers) / 16.0
        else:
            rate = 20.0 / 96.0
        attn_pair(j, p, pin3 if j == 3 else pin2, rate)
    while deferred:
        deferred.popleft()()
    while fillers:
        fillers.popleft()()
    for u in proj_units(3, pin3):
        u()


# ======================= host side =======================

_NC_CACHE = None
LAST_RESULT = None


def _get_program():
    global _NC_CACHE
    if _NC_CACHE is None:
        _NC_CACHE = build_program()
    return _NC_CACHE


def shard_inputs(x, w_qkv, b_qkv, w_proj):
    x = np.asarray(x, dtype=np.float32)
    w_qkv = np.asarray(w_qkv, dtype=np.float32)
    b_qkv = np.asarray(b_qkv, dtype=np.float32)
    w_proj = np.asarray(w_proj, dtype=np.float32)
    ones_const = np.ones((128, 64), dtype=np.float16)
    in_maps = []
    for c in range(NCORES):
        b = c % B
        half = c // B
        hs = half * (HPC * HD)  # 512
        wq = w_qkv[:, 0 * D + hs:0 * D + hs + HPC * HD]
        wk = w_qkv[:, 1 * D + hs:1 * D + hs + HPC * HD]
        wv_ = w_qkv[:, 2 * D + hs:2 * D + hs + HPC * HD]
        bq = b_qkv[0 * D + hs:0 * D + hs + HPC * HD]
        bk = b_qkv[1 * D + hs:1 * D + hs + HPC * HD]
        bv_ = b_qkv[2 * D + hs:2 * D + hs + HPC * HD]
        in_maps.append({
            "xT": np.ascontiguousarray(x[b].T).astype(np.float16),
            "wqk": np.ascontiguousarray(
                np.concatenate([wq, wk], axis=1)).astype(np.float16),
            "wv": np.ascontiguousarray(wv_).astype(np.float16),
            "wp": np.ascontiguousarray(w_proj[hs:hs + HPC * HD, :]).astype(np.float16),
            "bqk": np.ascontiguousarray(
                np.concatenate([bq, bk])[:, None]).astype(np.float32),
            "bv": np.ascontiguousarray(
                np.broadcast_to(bv_[None, :], (128, HPC * HD))).astype(np.float16),
            "ones": ones_const,
        })
    return in_maps


def kernel(x, w_qkv, b_qkv, w_proj, b_proj):
    global LAST_RESULT
    from concourse.bass_utils import run_bass_kernel_spmd

    nc = _get_program()
    in_maps = shard_inputs(x, w_qkv, b_qkv, w_proj)
    res = run_bass_kernel_spmd(nc, in_maps, list(range(NCORES)))
    LAST_RESULT = res
    b_proj = np.asarray(b_proj, dtype=np.float32)
    y = np.empty((B, T, D), dtype=np.float32)
    for b in range(B):
        yTfull = (res.results[b]["yT"].astype(np.float32)
                  + res.results[b + B]["yT"].astype(np.float32))
        y[b] = yTfull.T + b_proj[None, :]
    return y


# revision 10
# speedup vs baseline: 1.8929x; 1.0329x over previous
"""Causal self-attention (B=4, T=2048, D=1024, H=16, hd=64) on 8 TRN2 NeuronCores.

Sharding: core c handles batch b = c % 4 and head-half = c // 4 (8 heads each).
Each core computes, for its (batch, 8 heads):
    qkv projection -> causal attention -> partial output projection (yT).
Host gathers: y[b] = (yT[core b] + yT[core b+4]).T + b_proj.

Device design (per core), v2 (fp16):
  - everything fp16 on the PE (1 cyc/row, FWL weight loads); PSUM fp32
  - transposed layouts: xT [D, T], qT/kT [2heads x 64, pair, T], V [t, kt, 8h*64]
  - S^T [k, q] tiles: two heads row-packed at tile_position (0,0)/(64,0)
  - exp on ScalarE (scale fused); causal mask via gpsimd affine_select on
    diagonal tiles only
  - PV col-packed: head A -> psum rows 0:64 (tile_position (0,0)), head B ->
    rows 64:128 ((0,64)) - both matmuls run concurrently (one 512-cyc slot)
  - softmax denominator: DVE accumulates e tiles into a per-pair partial
    (fp16, 4x DVE mode); one col-packed pair of ones[128,64] matmuls reduces
    AND broadcasts the denominators into a [128,512] psum; DVE reciprocal ->
    bc; the PV psum drain is fused with the normalize multiply
  - emission interleaves p1/proj matmul "filler" units into the attention
    k-tile loop (engines are FIFO; emission order = execution order), with
    PV/denominator ops deferred by 2 steps so the PE never waits on exp
  - PE warmup matmuls + split initial DMAs (both HWDGE queues) cover the
    startup weight-load window and keep the HAM clock gate open
"""

import sys
from collections import deque
from contextlib import ExitStack

import numpy as np

sys.path.insert(0, "/opt/trn_rl_repo")

import concourse.bass as bass  # noqa: E402
import concourse.tile as tile  # noqa: E402
from concourse import bacc, mybir  # noqa: E402

f16 = mybir.dt.float16
bf16 = mybir.dt.bfloat16
f32 = mybir.dt.float32
EXP = mybir.ActivationFunctionType.Exp
ADD = mybir.AluOpType.add
MULT = mybir.AluOpType.mult

B, T, D = 4, 2048, 1024
H, HD = 16, 64
HPC = 8            # heads per core
NP = 4             # head pairs per core
NCORES = 8
TCH = 512          # p1 t-chunk width == attention q-chunk width
NCH = T // TCH     # 4
QCH = 512
NKT = T // 128     # 16 k-tiles


def build_program():
    nc = bacc.Bacc("TRN2", target_bir_lowering=False, debug=False)

    xT = nc.dram_tensor("xT", [D, T], f16, kind="ExternalInput").ap()
    wqk = nc.dram_tensor("wqk", [D, 2 * HPC * HD], f16, kind="ExternalInput").ap()
    wv = nc.dram_tensor("wv", [D, HPC * HD], f16, kind="ExternalInput").ap()
    wp = nc.dram_tensor("wp", [HPC * HD, D], f16, kind="ExternalInput").ap()
    bqk = nc.dram_tensor("bqk", [2 * HPC * HD, 1], f32, kind="ExternalInput").ap()
    bv = nc.dram_tensor("bv", [128, HPC * HD], f16, kind="ExternalInput").ap()
    yT = nc.dram_tensor("yT", [D, T], f16, kind="ExternalOutput").ap()

    with tile.TileContext(nc) as tc:
        with ExitStack() as ctx:
            _build(ctx, tc, xT, wqk, wv, wp, bqk, bv, yT)
    nc.compile()
    return nc


def _build(ctx, tc, xT, wqk, wv, wp, bqk, bv, yT):
    nc = tc.nc

    persist = ctx.enter_context(tc.tile_pool(name="persist", bufs=1))
    w_pool = ctx.enter_context(tc.tile_pool(name="w_pool", bufs=1))
    xc_pool = ctx.enter_context(tc.tile_pool(name="xc_pool", bufs=2))
    e_pool = ctx.enter_context(tc.tile_pool(name="e_pool", bufs=6))
    pa_pool = ctx.enter_context(tc.tile_pool(name="pa_pool", bufs=4))
    pin_pool = ctx.enter_context(tc.tile_pool(name="pin_pool", bufs=4))
    bc_pool = ctx.enter_context(tc.tile_pool(name="bc_pool", bufs=2))
    ysb_pool = ctx.enter_context(tc.tile_pool(name="ysb_pool", bufs=4))
    small = ctx.enter_context(tc.tile_pool(name="small", bufs=1))

    p1ps = ctx.enter_context(tc.tile_pool(name="p1ps", bufs=2, space="PSUM"))
    s_ps = ctx.enter_context(tc.tile_pool(name="s_ps", bufs=2, space="PSUM"))
    pv_ps = ctx.enter_context(tc.tile_pool(name="pv_ps", bufs=2, space="PSUM"))

    # ---- persistent tensors ----
    qT = persist.tile([128, NP, T], f16, tag="qT")      # [2 heads x 64 dims, pair, t]
    kT = persist.tile([128, NP, T], f16, tag="kT")
    V = persist.tile([128, NKT, HPC * HD], bf16, tag="V")  # [t in tile, k-tile, h*64+d]

    # ---- initial DMAs, round-robin over both HWDGE queues (sync + scalar) ----
    dmae = [nc.sync, nc.scalar]
    ones_sb = small.tile([128, 64], bf16, tag="ones_sb", bufs=1)
    nc.vector.memset(ones_sb, 1.0)

    xTr = xT.rearrange("(d p) t -> p d t", p=128)

    def load_xc(c):
        xc = xc_pool.tile([128, 8, TCH], f16, name=f"xc{c}", tag="xc")
        nc.sync.dma_start(out=xc[:, 0:4, :], in_=xTr[:, 0:4, c * TCH:(c + 1) * TCH])
        nc.scalar.dma_start(out=xc[:, 4:8, :], in_=xTr[:, 4:8, c * TCH:(c + 1) * TCH])
        return xc

    xc0 = load_xc(0)
    wqk_sb = w_pool.tile([128, 8, 2 * HPC * HD], f16, tag="wqk_sb")
    for i in range(8):
        dmae[i % 2].dma_start(out=wqk_sb[:, i, :], in_=wqk[i * 128:(i + 1) * 128, :])
    wv_sb = w_pool.tile([128, 8, HPC * HD], f16, tag="wv_sb")
    for i in range(8):
        nc.gpsimd.dma_start(out=wv_sb[:, i, :], in_=wv[i * 128:(i + 1) * 128, :])
    bqk_sb = small.tile([128, 8], f32, tag="bqk_sb", bufs=1)
    nc.gpsimd.dma_start(out=bqk_sb, in_=bqk.rearrange("(m p) o -> p (m o)", p=128))
    bv_sb = small.tile([128, HPC * HD], f16, tag="bv_sb", bufs=1)
    nc.gpsimd.dma_start(out=bv_sb, in_=bv)
    xc1 = load_xc(1)
    wp_sb = w_pool.tile([128, NP, D], f16, tag="wp_sb")
    for i in range(NP):
        dmae[i % 2].dma_start(out=wp_sb[:, i, :], in_=wp[i * 128:(i + 1) * 128, :])

    zreg = nc.gpsimd.to_reg(0.0)

    # ---- PE warmup: keep the PE busy (and the HAM gate open) during the
    # initial weight DMA; writes a never-read scratch psum ----
    scr = p1ps.tile([128, TCH], f32, name="warm", tag="p1")
    for i in range(136):
        nc.tensor.matmul(scr[0:64, 0:64], ones_sb, ones_sb, start=True, stop=True)

    # ================= phase 1 units (qkv projection) =================
    def p1_qk_unit(c, mt, xc):
        qk_ps = p1ps.tile([128, TCH], f32, name=f"qk{c}_{mt}", tag="p1")
        for dt in range(8):
            nc.tensor.matmul(qk_ps, wqk_sb[:, dt, mt * 128:(mt + 1) * 128],
                             xc[:, dt, :], start=(dt == 0), stop=(dt == 7))
        dest = qT if mt < 4 else kT
        nc.vector.tensor_scalar_add(
            dest[:, mt % 4, c * TCH:(c + 1) * TCH], qk_ps, bqk_sb[:, mt:mt + 1])

    def p1_v_unit(c, tt, xc):
        v_ps = p1ps.tile([128, 512], f32, name=f"v{c}_{tt}", tag="p1")
        for dt in range(8):
            nc.tensor.matmul(v_ps, xc[:, dt, tt * 128:(tt + 1) * 128],
                             wv_sb[:, dt, :], start=(dt == 0), stop=(dt == 7))
        nc.vector.tensor_tensor(out=V[:, c * 4 + tt, :], in0=v_ps, in1=bv_sb, op=ADD)

    def p1_units(c, xc):
        units = [(lambda c=c, mt=mt, xc=xc: p1_qk_unit(c, mt, xc)) for mt in range(8)]
        units += [(lambda c=c, tt=tt, xc=xc: p1_v_unit(c, tt, xc)) for tt in range(4)]
        return units

    # ================= phase 3 unit (output projection) =================
    def proj_unit(j, pin, mt):
        y_ps = p1ps.tile([128, QCH], f32, name=f"y{j}_{mt}", tag="p1")
        for p in range(NP):
            nc.tensor.matmul(y_ps, wp_sb[:, p, mt * 128:(mt + 1) * 128],
                             pin[:, p, :], start=(p == 0), stop=(p == NP - 1))
        y_sb = ysb_pool.tile([128, QCH], f16, name=f"ysb{j}_{mt}", tag="ysb")
        nc.vector.tensor_copy(out=y_sb, in_=y_ps)
        nc.sync.dma_start(
            out=yT[mt * 128:(mt + 1) * 128, j * QCH:(j + 1) * QCH], in_=y_sb)

    def interleave(a, b):
        out = []
        la, lb = list(a), list(b)
        n = max(len(la), len(lb))
        for i in range(n):
            if i < len(la):
                out.append(la[i])
            if i < len(lb):
                out.append(lb[i])
        return out

    def proj_units(j, pin):
        return [(lambda j=j, pin=pin, mt=mt: proj_unit(j, pin, mt)) for mt in range(8)]

    # ================= phase 2: attention =================
    # deferred: PE-op closures executed with a 2-step lag so the PE (FIFO)
    # never queues a matmul whose exp input isn't ready yet.
    deferred = deque()
    fillers = deque()
    fill_state = {"acc": 0.0}

    def attn_pair(j, p, pin, rate):
        nkt = 4 * (j + 1)
        q0 = j * QCH
        partial = pa_pool.tile([128, 2 * QCH], bf16, name=f"pa{j}_{p}", tag="pa")
        pv = pv_ps.tile([128, QCH], f32, name=f"pv{j}_{p}", tag="pv")
        for kt in range(nkt):
            o = max(0, kt - 4 * j)
            c0 = 128 * o
            W = QCH - c0
            s = s_ps.tile([128, 2 * QCH], f32, name=f"s{j}_{p}_{kt}", tag="s")
            nc.tensor.matmul(
                s[:, c0:QCH], kT[0:64, p, kt * 128:(kt + 1) * 128],
                qT[0:64, p, q0 + c0:q0 + QCH],
                start=True, stop=True, tile_position=(0, 0))
            nc.tensor.matmul(
                s[:, QCH + c0:2 * QCH], kT[64:128, p, kt * 128:(kt + 1) * 128],
                qT[64:128, p, q0 + c0:q0 + QCH],
                start=True, stop=True, tile_position=(64, 0))
            e = e_pool.tile([128, 2 * QCH], bf16, name=f"e{j}_{p}_{kt}", tag="e")
            if o == 0:
                nc.scalar.activation(e, s, EXP, scale=0.125)
            else:
                sv = s.rearrange("p (h q) -> p h q", h=2)[:, :, c0:QCH]
                ev = e.rearrange("p (h q) -> p h q", h=2)[:, :, c0:QCH]
                nc.scalar.activation(ev, sv, EXP, scale=0.125)
            if kt >= 4 * j:
                # staircase mask within the remaining width: keep col >= k
                ev2 = e.rearrange("p (h q) -> p h q", h=2)[:, :, c0:QCH]
                nc.gpsimd.affine_select(
                    ev2, ev2, pattern=[[0, 2], [1, W]],
                    compare_op=mybir.AluOpType.is_ge, fill=zreg,
                    base=0, channel_multiplier=-1)
            # denominator partials (DVE)
            if kt == 0:
                nc.vector.tensor_copy(out=partial, in_=e)
            elif o == 0:
                nc.vector.tensor_tensor(out=partial, in0=partial, in1=e, op=ADD)
            else:
                pview = partial.rearrange("p (h q) -> p h q", h=2)[:, :, c0:QCH]
                eview = e.rearrange("p (h q) -> p h q", h=2)[:, :, c0:QCH]
                nc.vector.tensor_tensor(out=pview, in0=pview, in1=eview, op=ADD)

            def emit_pv(p=p, kt=kt, c0=c0, e=e, pv=pv, last=(kt == nkt - 1)):
                nc.tensor.matmul(
                    pv[0:64, c0:QCH], V[:, kt, (2 * p) * 64:(2 * p + 1) * 64],
                    e[:, c0:QCH], start=(kt == 0), stop=last)
                nc.tensor.matmul(
                    pv[64:128, c0:QCH], V[:, kt, (2 * p + 1) * 64:(2 * p + 2) * 64],
                    e[:, QCH + c0:2 * QCH], start=(kt == 0), stop=last)

            deferred.append(emit_pv)
            while len(deferred) > 2:
                deferred.popleft()()
            fill_state["acc"] += rate
            while fillers and fill_state["acc"] >= 1.0:
                fillers.popleft()()
                fill_state["acc"] -= 1.0

        def emit_fin(j=j, p=p, partial=partial, pv=pv, pin=pin):
            # ones[128,64] matmuls: reduce the partial over k AND broadcast
            # the per-head denominators into all 64 rows of a psum bank
            dn = p1ps.tile([128, QCH], f32, name=f"dn{j}_{p}", tag="p1")
            nc.tensor.matmul(dn[0:64, :], ones_sb, partial[:, 0:QCH],
                             start=True, stop=True)
            nc.tensor.matmul(dn[64:128, :], ones_sb, partial[:, QCH:2 * QCH],
                             start=True, stop=True)
            bc = bc_pool.tile([128, QCH], f32, name=f"bc{j}_{p}", tag="bc")
            nc.vector.reciprocal_approx_fast(out=bc, in_=dn)
            # fused PV-psum drain + normalize
            nc.vector.tensor_tensor(out=pin[:, p, :], in0=pv, in1=bc, op=MULT)

        deferred.append(emit_fin)

    # ================= emission =================
    for mt in range(8):
        p1_qk_unit(0, mt, xc0)
    for tt in range(4):
        p1_v_unit(0, tt, xc0)

    xc_next = {}

    def prefetch(c):
        xc_next[c] = load_xc(c)

    def new_pin(j):
        return pin_pool.tile([128, NP, QCH], f16, name=f"pin{j}", tag="pin")

    # --- chunk 0: fillers = p1(c1) ---
    prefetch(2)
    pin0 = new_pin(0)
    fillers.extend(p1_units(1, xc1))
    fill_state["acc"] = 0.0
    for p in range(NP):
        attn_pair(0, p, pin0, 12.0 / 16.0)
    while fillers:
        fillers.popleft()()

    # --- chunk 1: fillers = p1 qk(c2), V(c2), qk(c3) ---
    prefetch(3)
    pin1 = new_pin(1)
    xc2, xc3 = xc_next[2], xc_next[3]
    fillers.extend([(lambda mt=mt: p1_qk_unit(2, mt, xc2)) for mt in range(8)])
    fillers.extend([(lambda tt=tt: p1_v_unit(2, tt, xc2)) for tt in range(4)])
    fillers.extend([(lambda mt=mt: p1_qk_unit(3, mt, xc3)) for mt in range(8)])
    fill_state["acc"] = 0.0
    for p in range(NP):
        attn_pair(1, p, pin1, 20.0 / 32.0)
    while fillers:
        fillers.popleft()()

    # --- chunks 2+3 interleaved by pair: exp-heavy chunk-3 pairs overlap
    # proj filler work instead of piling up at the end ---
    pin2 = new_pin(2)
    pin3 = new_pin(3)
    fillers.extend([(lambda tt=tt: p1_v_unit(3, tt, xc3)) for tt in range(4)])
    fillers.extend(interleave(proj_units(0, pin0), proj_units(1, pin1)))
    fill_state["acc"] = 0.0
    region = [(2, 0), (3, 0), (2, 1), (3, 1), (2, 2), (3, 2), (2, 3), (3, 3)]
    for idx, (j, p) in enumerate(region):
        if (j, p) == (3, 3):
            fillers.extend(proj_units(2, pin2))
            fill_state["acc"] = -2.0 * (len(fillers) / 16.0)
            rate = len(fillers) / 16.0
        else:
            rate = 20.0 / 96.0
        attn_pair(j, p, pin3 if j == 3 else pin2, rate)
    while deferred:
        deferred.popleft()()
    while fillers:
        fillers.popleft()()
    for u in proj_units(3, pin3):
        u()


# ======================= host side =======================

_NC_CACHE = None
LAST_RESULT = None


def _get_program():
    global _NC_CACHE
    if _NC_CACHE is None:
        _NC_CACHE = build_program()
    return _NC_CACHE


def shard_inputs(x, w_qkv, b_qkv, w_proj):
    x = np.asarray(x, dtype=np.float32)
    w_qkv = np.asarray(w_qkv, dtype=np.float32)
    b_qkv = np.asarray(b_qkv, dtype=np.float32)
    w_proj = np.asarray(w_proj, dtype=np.float32)
    ones_const = np.ones((128, 64), dtype=np.float16)
    in_maps = []
    for c in range(NCORES):
        b = c % B
        half = c // B
        hs = half * (HPC * HD)  # 512
        wq = w_qkv[:, 0 * D + hs:0 * D + hs + HPC * HD]
        wk = w_qkv[:, 1 * D + hs:1 * D + hs + HPC * HD]
        wv_ = w_qkv[:, 2 * D + hs:2 * D + hs + HPC * HD]
        bq = b_qkv[0 * D + hs:0 * D + hs + HPC * HD]
        bk = b_qkv[1 * D + hs:1 * D + hs + HPC * HD]
        bv_ = b_qkv[2 * D + hs:2 * D + hs + HPC * HD]
        in_maps.append({
            "xT": np.ascontiguousarray(x[b].T).astype(np.float16),
            "wqk": np.ascontiguousarray(
                np.concatenate([wq, wk], axis=1)).astype(np.float16),
            "wv": np.ascontiguousarray(wv_).astype(np.float16),
            "wp": np.ascontiguousarray(w_proj[hs:hs + HPC * HD, :]).astype(np.float16),
            "bqk": np.ascontiguousarray(
                np.concatenate([bq, bk])[:, None]).astype(np.float32),
            "bv": np.ascontiguousarray(
                np.broadcast_to(bv_[None, :], (128, HPC * HD))).astype(np.float16),
            "ones": ones_const,
        })
    return in_maps


def kernel(x, w_qkv, b_qkv, w_proj, b_proj):
    global LAST_RESULT
    from concourse.bass_utils import run_bass_kernel_spmd

    nc = _get_program()
    in_maps = shard_inputs(x, w_qkv, b_qkv, w_proj)
    res = run_bass_kernel_spmd(nc, in_maps, list(range(NCORES)))
    LAST_RESULT = res
    b_proj = np.asarray(b_proj, dtype=np.float32)
    y = np.empty((B, T, D), dtype=np.float32)
    for b in range(B):
        yTfull = (res.results[b]["yT"].astype(np.float32)
                  + res.results[b + B]["yT"].astype(np.float32))
        y[b] = yTfull.T + b_proj[None, :]
    return y
